# revision 5
# baseline (speedup 1.0000x reference)
"""Trainium2 Bass kernel for hierarchical (sibling-group) softmax over
hyperplane margins.

For x:(8,64,128,128), normals/offsets:(1024,64), sibmat block-diagonal with
32-wide sibling groups:

    logits[b,m,h,w] = <x[b,:,h,w], normals[m]> - <normals[m], offsets[m]>
    out = exp(logits) / (group_sum_32(exp(logits)) + 1e-15)

Sharding: data-parallel over batch, one batch element per NeuronCore (8
cores), no collectives.  Per core, m-chunks of 128 rows live on partitions
and pixels on the free axis.

Design (v2, measured on HW):
  1. PE matmul fp16: logits = normals.T @ x into fp32 PSUM, 4 x N=512 MMs
     per (superblock, chunk) sharing one LDWEIGHTS.
  2. ACT exp PSUM->SBUF fp16 (N=1024 per instruction), per-partition bias
     supplies -<normals_m, offsets_m> in fp32.
  3. PE matmul with per-chunk [128, 32] group indicators accumulates Z for
     4 blocks into one [128, 1024] PSUM tile; the 4 blocks' matmuls use
     col-tile_position 32*bg, so quads execute concurrently in the array
     (~3.7x measured).
  4. DVE reciprocal PSUM->SBUF fp16 (column-quarters to keep the in-order
     DVE stream responsive).
  5. The 1/Z rows are replicated 32x across partitions by SBUF->SBUF DMAs
     whose source AP has a zero-step middle dim (4 rows -> 128 partitions);
     this keeps both operands of the final multiply fp16 in SBUF, so the
     DVE tensor_tensor runs in 2x packed mode (vs 1x reading fp32 PSUM).
  6. DVE multiply e * (1/Z) -> fp16 [128, 2048] tiles, DMAed straight into
     the (M, H, W) output layout; the host upcasts fp16 -> fp32.

All PE work is emitted in long accumulation chains with few semaphore
waits so the HAM clock-gate reaches the warm 2.4 GHz state (cold-start
kernels measure 1.2 GHz).  A post-pass splits multi-wait instructions
(walrus's TRN2 codegen encodes at most one semaphore wait per compute
instruction).  fp16 keeps all value ranges exact to ~1.5e-3 of the fp32
reference (guarded by input-range checks that fall back to an exact host
implementation).
"""

import numpy as np

B, D, H, W = 8, 64, 128, 128
M = 1024
GROUP = 32
PIX = H * W          # 16384 pixels per batch element
BLK = 1024           # pixels per block
SBW = 2048           # pixels per superblock (2 blocks)
NSB = PIX // SBW     # 8 superblocks
MC = 128             # m-chunk width (partition dim)
NCHUNK = M // MC     # 8
NCORES = 8
FMAX = 512           # max moving free dim into one PSUM bank (fp32 out)
NSG = 4              # supergroups (4 blocks each) per core

_cache = {}


_WAIT_OK_OPCODES = {"Call"}


def _split_excess_waits(nc):
    """Walrus's TRN2 codegen (CoreV3GenImpl setupSyncWait) encodes at most
    one semaphore wait per compute instruction (Matmult, TensorTensor, ...);
    Tile can legitimately attach several (e.g. waits on two input DMAs).
    Move all but one wait onto EventSemaphore instructions inserted just
    before the instruction on the same engine — ordering is identical."""
    import concourse.mybir as mybir

    n_fixed = 0
    for f in nc.m.functions:
        for blk in f.blocks:
            out = []
            changed = False
            for inst in blk.instructions:
                si = inst.sync_info
                if (
                    si is not None
                    and len(si.on_wait) > 1
                    and inst.opcode not in _WAIT_OK_OPCODES
                ):
                    waits = list(si.on_wait)
                    for j, w in enumerate(waits[:-1]):
                        out.append(
                            mybir.InstEventSemaphore(
                                name=f"{inst.name}-wsplit{j}",
                                opcode="EventSemaphore",
                                engine=inst.engine,
                                sync_info=mybir.SyncInfo(
                                    on_wait=[w], on_update=[]
                                ),
                            )
                        )
                    inst.sync_info = mybir.SyncInfo(
                        on_wait=[waits[-1]], on_update=list(si.on_update)
                    )
                    changed = True
                    n_fixed += 1
                out.append(inst)
            if changed:
                blk.instructions = out
    return n_fixed


def _build_nc():
    import concourse.bass as bass
    import concourse.mybir as mybir
    import concourse.tile as tile

    f32 = mybir.dt.float32
    f16 = mybir.dt.float16
    nc = bass.Bass()

    x_in = nc.declare_dram_parameter("x_bf", [D, PIX], f16, isOutput=False)
    w_in = nc.declare_dram_parameter("normals_bf", [D, M], f16, isOutput=False)
    # gsum_w[:, mc*32:(mc+1)*32]: [128, 32] indicator, [p, r] = 1 iff
    # r == 4*mc + p//32 — maps chunk mc's partitions to its 32 global
    # groups-within-block rows of the Z tile.
    g_in = nc.declare_dram_parameter("gsum_w", [MC, NCHUNK * 32], f16,
                                     isOutput=False)
    # bias_neg[p, mc] = -<normals, offsets> for m = mc*128+p; applied as the
    # ACT exp per-partition bias (exact fp32).
    c_in = nc.declare_dram_parameter("bias_neg", [MC, NCHUNK], f32,
                                     isOutput=False)
    y_out = nc.declare_dram_parameter("y", [M, PIX], f16, isOutput=True)

    with tile.TileContext(nc) as tc:
        with (
            tc.tile_pool(name="const", bufs=1) as cpool,
            tc.tile_pool(name="xin", bufs=3) as xpool,
            tc.tile_pool(name="expv", bufs=12) as epool,
            tc.tile_pool(name="recv", bufs=2) as rpool,
            tc.tile_pool(name="rbv", bufs=5) as rbpool,
            tc.tile_pool(name="outv", bufs=4) as opool,
            tc.tile_pool(name="psl", bufs=3, space="PSUM") as pslp,
            tc.tile_pool(name="psz", bufs=1, space="PSUM") as pszp,
        ):
            w_sb = cpool.tile([D, M], f16)
            nc.sync.dma_start(w_sb[:], w_in[:])
            g_sb = cpool.tile([MC, NCHUNK * 32], f16)
            nc.sync.dma_start(g_sb[:], g_in[:])
            c_sb = cpool.tile([MC, NCHUNK], f32)
            nc.sync.dma_start(c_sb[:], c_in[:])

            x_of = {}
            e_of = {}
            rec_of = {}
            psz_of = {}

            def fetch_x(sb):
                if sb in x_of or sb >= NSB:
                    return
                x_t = xpool.tile([D, SBW], f16, tag="x_t", name="x_t")
                nc.sync.dma_start(x_t[:], x_in[:, sb * SBW:(sb + 1) * SBW])
                x_of[sb] = x_t

            def mm1_exp(sb, mc):
                """logits + exp for chunk mc of superblock sb (2 blocks).
                e tiles span a whole supergroup [128, 4096] so the tail
                (broadcast/multiply/out-DMA) works in coarse units."""
                x_t = x_of[sb]
                sg = sb // 2
                if sb % 2 == 0:
                    e_of[sg, mc] = epool.tile([MC, 2 * SBW], f16,
                                              tag="e_t", name="e_t")
                e_t = e_of[sg, mc]
                for b in range(2):          # block within superblock
                    cb = (sb % 2) * SBW + b * BLK
                    ps = pslp.tile([MC, BLK], f32, tag="ps_l", name="ps_l")
                    for h in range(2):
                        nc.tensor.matmul(
                            ps[:, h * FMAX:(h + 1) * FMAX],
                            w_sb[:, mc * MC:(mc + 1) * MC],
                            x_t[:, b * BLK + h * FMAX:
                                b * BLK + (h + 1) * FMAX],
                            start=True, stop=True,
                        )
                    nc.scalar.activation(
                        e_t[:, cb:cb + BLK], ps[:],
                        mybir.ActivationFunctionType.Exp,
                        bias=c_sb[:, mc:mc + 1],
                    )

            def mm2_batch(sg):
                """Z for supergroup sg (blocks 4sg..4sg+3) into one
                [128, 1024] PSUM tile; 4 blocks via col-tile_position run
                concurrently in the PE array."""
                ps_z = pszp.tile([4 * 32, BLK], f32, tag="ps_z", name="ps_z")
                psz_of[sg] = ps_z
                for half in range(2):
                    for mc in range(NCHUNK):
                        for bgl in range(4):
                            e_t = e_of[sg, mc]
                            nc.tensor.matmul(
                                ps_z[32 * bgl:32 * (bgl + 1),
                                     half * FMAX:(half + 1) * FMAX],
                                g_sb[:, mc * 32:(mc + 1) * 32],
                                e_t[:, bgl * BLK + half * FMAX:
                                    bgl * BLK + (half + 1) * FMAX],
                                start=(mc == 0), stop=(mc == NCHUNK - 1),
                                tile_position=(0, 32 * bgl),
                                skip_group_check=True,
                            )

            def emit_recip(sg, q):
                QW = BLK // 4
                if q == 0:
                    rec_of[sg] = rpool.tile([4 * 32, BLK], f16, tag="rec",
                                            name="rec")
                rec = rec_of[sg]
                ps_z = psz_of[sg]
                with nc.allow_low_precision(
                    reason="fp16 rounding of reciprocal feeding the "
                    "broadcast; well within output tolerance"
                ):
                    nc.vector.reciprocal(
                        rec[:, q * QW:(q + 1) * QW],
                        ps_z[:, q * QW:(q + 1) * QW],
                    )
                if q == 3:
                    del psz_of[sg]

            rb_of = {}

            def emit_bcast(sg, mc):
                """Replicate 1/Z rows 32x across partitions for (sg, mc).
                SWDGE (GpSimd queue) issues these at ~0.8us vs ~6us on the
                HWDGE Sync queue; emitted a few units ahead of the multiply
                so the waits are pre-satisfied."""
                rec = rec_of[sg]
                rb = rbpool.tile([MC, 2 * SBW], f16, tag="rb", name="rb")
                for bgl in range(4):
                    rows = rec[32 * bgl + 4 * mc:32 * bgl + 4 * mc + 4, :]
                    nc.gpsimd.dma_start(
                        rb[:, bgl * BLK:(bgl + 1) * BLK],
                        rows.unsqueeze(1).broadcast_to([4, 32, BLK]),
                    )
                rb_of[sg, mc] = rb

            outq = []

            def mul_unit(sg, mc):
                """multiply for (sg, mc); the output DMA is emitted two
                units later (Sync queue) so its wait-on-mul is pre-satisfied
                and never head-blocks the queue."""
                e_t = e_of.pop((sg, mc))
                rb = rb_of.pop((sg, mc))
                o_t = opool.tile([MC, 2 * SBW], f16, tag="o_t", name="o_t")
                nc.vector.tensor_mul(o_t[:], e_t[:], rb[:])
                outq.append((sg, mc, o_t))
                while len(outq) > 2:
                    emit_out()

            def emit_out():
                sg, mc, o_t = outq.pop(0)
                nc.sync.dma_start(
                    y_out[mc * MC:(mc + 1) * MC,
                          sg * 2 * SBW:(sg + 1) * 2 * SBW],
                    o_t[:],
                )

            pending = []
            fetch_x(0)
            fetch_x(1)
            DRAIN_PER_SLOT = 5
            BCAST_AHEAD = 3

            def drain(n):
                for _ in range(min(n, len(pending))):
                    for k in range(min(BCAST_AHEAD, len(pending))):
                        u = pending[k]
                        if u not in rb_of and u[0] in rec_of:
                            emit_bcast(*u)
                    u = pending.pop(0)
                    if u not in rb_of:
                        emit_bcast(*u)
                    mul_unit(*u)

            for j in range(NSB + 3):
                if j < NSB:
                    fetch_x(j + 2)
                    for mc in range(NCHUNK):
                        mm1_exp(j, mc)
                if j <= NSB and j % 2 == 1:
                    sg = (j - 1) // 2
                    mm2_batch(sg)
                    for q in range(4):
                        emit_recip(sg, q)
                    for mc in range(NCHUNK):
                        pending.append((sg, mc))
                drain(DRAIN_PER_SLOT)
            drain(len(pending))
            assert not pending, len(pending)
            while outq:
                emit_out()

    _split_excess_waits(nc)
    return nc


def _prep_core_inputs(x, normals, offsets):
    f16 = np.float16
    bias = np.einsum("md,md->m", normals, offsets).astype(np.float32)
    w_bf = np.ascontiguousarray(normals.T).astype(f16)

    gid = np.arange(M) // GROUP                     # global group of each m
    gsum = np.zeros((MC, NCHUNK * 32), np.float32)
    for mc in range(NCHUNK):
        for p in range(MC):
            r = gid[mc * MC + p] % 32               # group-within-block row
            gsum[p, mc * 32 + r] = 1.0
    gsum = gsum.astype(f16)
    bias_neg = np.ascontiguousarray(
        -bias.reshape(NCHUNK, MC).T
    ).astype(np.float32)

    in_maps = []
    for b in range(NCORES):
        x_bf = np.ascontiguousarray(x[b].reshape(D, PIX)).astype(f16)
        in_maps.append(
            {"x_bf": x_bf, "normals_bf": w_bf, "gsum_w": gsum,
             "bias_neg": bias_neg}
        )
    return in_maps


def _sibmat_is_expected(sibmat):
    gid = np.arange(M) // GROUP
    expected = (gid[:, None] == gid[None, :]).astype(np.float32)
    return sibmat.shape == (M, M) and np.array_equal(sibmat, expected)


def _numpy_fallback(x, normals, offsets, sibmat):
    # Straight fp32 transcription of the reference; only used if sibmat is
    # not the expected 32-wide block-diagonal matrix.
    bias = np.einsum("md,md->m", normals, offsets)
    out = np.empty((B, M, H, W), np.float32)
    for b in range(B):
        logits = np.einsum("dhw,md->mhw", x[b], normals) - bias[:, None, None]
        logits -= np.max(logits, axis=0, keepdims=True)
        e = np.exp(logits)
        z = np.einsum("mhw,nm->nhw", e, sibmat)
        out[b] = e / (z + 1e-15)
    return out


def kernel(x, normals, offsets, sibmat, steps=None, trace=False, **_ignored):
    from concourse.bass_utils import run_bass_kernel_spmd

    x = np.ascontiguousarray(np.asarray(x, dtype=np.float32))
    normals = np.ascontiguousarray(np.asarray(normals, dtype=np.float32))
    offsets = np.ascontiguousarray(np.asarray(offsets, dtype=np.float32))
    sibmat = np.ascontiguousarray(np.asarray(sibmat, dtype=np.float32))

    if (
        not _sibmat_is_expected(sibmat)
        or np.abs(normals).max() > 0.5
        or np.abs(x).max() > 12.0
    ):
        # unexpected structure or value ranges outside the fp16-safe
        # envelope of the device kernel: compute exactly on host
        return _numpy_fallback(x, normals, offsets, sibmat)

    if "nc" not in _cache:
        _cache["nc"] = _build_nc()
    nc = _cache["nc"]

    in_maps = _prep_core_inputs(x, normals, offsets)
    res = run_bass_kernel_spmd(nc, in_maps, list(range(NCORES)), trace=trace)
    out = np.stack(
        [np.asarray(r["y"]).astype(np.float32).reshape(M, H, W)
         for r in res.results]
    )
    kernel.last_result = res
    return out


# revision 7
# speedup vs baseline: 1.0032x; 1.0032x over previous
"""Trainium2 Bass kernel for hierarchical (sibling-group) softmax over
hyperplane margins.

For x:(8,64,128,128), normals/offsets:(1024,64), sibmat block-diagonal with
32-wide sibling groups:

    logits[b,m,h,w] = <x[b,:,h,w], normals[m]> - <normals[m], offsets[m]>
    out = exp(logits) / (group_sum_32(exp(logits)) + 1e-15)

Sharding: data-parallel over batch, one batch element per NeuronCore (8
cores), no collectives.  Per core, m-chunks of 128 rows live on partitions
and pixels on the free axis.

Design (v2, measured on HW):
  1. PE matmul fp16: logits = normals.T @ x into fp32 PSUM, 4 x N=512 MMs
     per (superblock, chunk) sharing one LDWEIGHTS.
  2. ACT exp PSUM->SBUF fp16 (N=1024 per instruction), per-partition bias
     supplies -<normals_m, offsets_m> in fp32.
  3. PE matmul with per-chunk [128, 32] group indicators accumulates Z for
     4 blocks into one [128, 1024] PSUM tile; the 4 blocks' matmuls use
     col-tile_position 32*bg, so quads execute concurrently in the array
     (~3.7x measured).
  4. DVE reciprocal PSUM->SBUF fp16 (column-quarters to keep the in-order
     DVE stream responsive).
  5. The 1/Z rows are replicated 32x across partitions by SBUF->SBUF DMAs
     whose source AP has a zero-step middle dim (4 rows -> 128 partitions);
     this keeps both operands of the final multiply fp16 in SBUF, so the
     DVE tensor_tensor runs in 2x packed mode (vs 1x reading fp32 PSUM).
  6. DVE multiply e * (1/Z) -> fp16 [128, 2048] tiles, DMAed straight into
     the (M, H, W) output layout; the host upcasts fp16 -> fp32.

All PE work is emitted in long accumulation chains with few semaphore
waits so the HAM clock-gate reaches the warm 2.4 GHz state (cold-start
kernels measure 1.2 GHz).  A post-pass splits multi-wait instructions
(walrus's TRN2 codegen encodes at most one semaphore wait per compute
instruction).  fp16 keeps all value ranges exact to ~1.5e-3 of the fp32
reference (guarded by input-range checks that fall back to an exact host
implementation).
"""

import numpy as np

B, D, H, W = 8, 64, 128, 128
M = 1024
GROUP = 32
PIX = H * W          # 16384 pixels per batch element
BLK = 1024           # pixels per block
SBW = 2048           # pixels per superblock (2 blocks)
NSB = PIX // SBW     # 8 superblocks
MC = 128             # m-chunk width (partition dim)
NCHUNK = M // MC     # 8
NCORES = 8
FMAX = 512           # max moving free dim into one PSUM bank (fp32 out)
NSG = 4              # supergroups (4 blocks each) per core

_cache = {}


_WAIT_OK_OPCODES = {"Call"}


def _split_excess_waits(nc):
    """Walrus's TRN2 codegen (CoreV3GenImpl setupSyncWait) encodes at most
    one semaphore wait per compute instruction (Matmult, TensorTensor, ...);
    Tile can legitimately attach several (e.g. waits on two input DMAs).
    Move all but one wait onto EventSemaphore instructions inserted just
    before the instruction on the same engine — ordering is identical."""
    import concourse.mybir as mybir

    n_fixed = 0
    for f in nc.m.functions:
        for blk in f.blocks:
            out = []
            changed = False
            for inst in blk.instructions:
                si = inst.sync_info
                if (
                    si is not None
                    and len(si.on_wait) > 1
                    and inst.opcode not in _WAIT_OK_OPCODES
                ):
                    waits = list(si.on_wait)
                    for j, w in enumerate(waits[:-1]):
                        out.append(
                            mybir.InstEventSemaphore(
                                name=f"{inst.name}-wsplit{j}",
                                opcode="EventSemaphore",
                                engine=inst.engine,
                                sync_info=mybir.SyncInfo(
                                    on_wait=[w], on_update=[]
                                ),
                            )
                        )
                    inst.sync_info = mybir.SyncInfo(
                        on_wait=[waits[-1]], on_update=list(si.on_update)
                    )
                    changed = True
                    n_fixed += 1
                out.append(inst)
            if changed:
                blk.instructions = out
    return n_fixed


def _build_nc():
    import concourse.bass as bass
    import concourse.mybir as mybir
    import concourse.tile as tile

    f32 = mybir.dt.float32
    f16 = mybir.dt.float16
    nc = bass.Bass()

    x_in = nc.declare_dram_parameter("x_bf", [D, PIX], f16, isOutput=False)
    w_in = nc.declare_dram_parameter("normals_bf", [D, M], f16, isOutput=False)
    # gsum_w[:, mc*32:(mc+1)*32]: [128, 32] indicator, [p, r] = 1 iff
    # r == 4*mc + p//32 — maps chunk mc's partitions to its 32 global
    # groups-within-block rows of the Z tile.
    g_in = nc.declare_dram_parameter("gsum_w", [MC, NCHUNK * 32], f16,
                                     isOutput=False)
    # bias_neg[p, mc] = -<normals, offsets> for m = mc*128+p; applied as the
    # ACT exp per-partition bias (exact fp32).
    c_in = nc.declare_dram_parameter("bias_neg", [MC, NCHUNK], f32,
                                     isOutput=False)
    y_out = nc.declare_dram_parameter("y", [M, PIX], f16, isOutput=True)

    with tile.TileContext(nc) as tc:
        with (
            tc.tile_pool(name="const", bufs=1) as cpool,
            tc.tile_pool(name="xin", bufs=3) as xpool,
            tc.tile_pool(name="expv", bufs=12) as epool,
            tc.tile_pool(name="recv", bufs=2) as rpool,
            tc.tile_pool(name="rbv", bufs=6) as rbpool,
            tc.tile_pool(name="outv", bufs=4) as opool,
            tc.tile_pool(name="psl", bufs=3, space="PSUM") as pslp,
            tc.tile_pool(name="psz", bufs=1, space="PSUM") as pszp,
        ):
            w_sb = cpool.tile([D, M], f16)
            nc.sync.dma_start(w_sb[:], w_in[:])
            g_sb = cpool.tile([MC, NCHUNK * 32], f16)
            nc.sync.dma_start(g_sb[:], g_in[:])
            c_sb = cpool.tile([MC, NCHUNK], f32)
            nc.sync.dma_start(c_sb[:], c_in[:])

            x_of = {}
            e_of = {}
            rec_of = {}
            psz_of = {}

            def fetch_x(sb):
                if sb in x_of or sb >= NSB:
                    return
                x_t = xpool.tile([D, SBW], f16, tag="x_t", name="x_t")
                nc.sync.dma_start(x_t[:], x_in[:, sb * SBW:(sb + 1) * SBW])
                x_of[sb] = x_t

            def mm1_exp(sb, mc):
                """logits + exp for chunk mc of superblock sb (2 blocks).
                e tiles span a whole supergroup [128, 4096] so the tail
                (broadcast/multiply/out-DMA) works in coarse units."""
                x_t = x_of[sb]
                sg = sb // 2
                if sb % 2 == 0:
                    e_of[sg, mc] = epool.tile([MC, 2 * SBW], f16,
                                              tag="e_t", name="e_t")
                e_t = e_of[sg, mc]
                for b in range(2):          # block within superblock
                    cb = (sb % 2) * SBW + b * BLK
                    ps = pslp.tile([MC, BLK], f32, tag="ps_l", name="ps_l")
                    for h in range(2):
                        nc.tensor.matmul(
                            ps[:, h * FMAX:(h + 1) * FMAX],
                            w_sb[:, mc * MC:(mc + 1) * MC],
                            x_t[:, b * BLK + h * FMAX:
                                b * BLK + (h + 1) * FMAX],
                            start=True, stop=True,
                        )
                    nc.scalar.activation(
                        e_t[:, cb:cb + BLK], ps[:],
                        mybir.ActivationFunctionType.Exp,
                        bias=c_sb[:, mc:mc + 1],
                    )

            def mm2_batch(sg):
                """Z for supergroup sg (blocks 4sg..4sg+3) into one
                [128, 1024] PSUM tile; 4 blocks via col-tile_position run
                concurrently in the PE array."""
                ps_z = pszp.tile([4 * 32, BLK], f32, tag="ps_z", name="ps_z")
                psz_of[sg] = ps_z
                for half in range(2):
                    for mc in range(NCHUNK):
                        for bgl in range(4):
                            e_t = e_of[sg, mc]
                            nc.tensor.matmul(
                                ps_z[32 * bgl:32 * (bgl + 1),
                                     half * FMAX:(half + 1) * FMAX],
                                g_sb[:, mc * 32:(mc + 1) * 32],
                                e_t[:, bgl * BLK + half * FMAX:
                                    bgl * BLK + (half + 1) * FMAX],
                                start=(mc == 0), stop=(mc == NCHUNK - 1),
                                tile_position=(0, 32 * bgl),
                                skip_group_check=True,
                            )

            def emit_recip(sg, q):
                QW = BLK // 4
                if q == 0:
                    rec_of[sg] = rpool.tile([4 * 32, BLK], f16, tag="rec",
                                            name="rec")
                rec = rec_of[sg]
                ps_z = psz_of[sg]
                with nc.allow_low_precision(
                    reason="fp16 rounding of reciprocal feeding the "
                    "broadcast; well within output tolerance"
                ):
                    nc.vector.reciprocal(
                        rec[:, q * QW:(q + 1) * QW],
                        ps_z[:, q * QW:(q + 1) * QW],
                    )
                if q == 3:
                    del psz_of[sg]

            rb_of = {}

            def emit_bcast(sg, mc):
                """Replicate 1/Z rows 32x across partitions for (sg, mc).
                SWDGE (GpSimd queue) issues these at ~0.8us vs ~6us on the
                HWDGE Sync queue; emitted a few units ahead of the multiply
                so the waits are pre-satisfied."""
                rec = rec_of[sg]
                rb = rbpool.tile([MC, 2 * SBW], f16, tag="rb", name="rb")
                for bgl in range(4):
                    rows = rec[32 * bgl + 4 * mc:32 * bgl + 4 * mc + 4, :]
                    nc.gpsimd.dma_start(
                        rb[:, bgl * BLK:(bgl + 1) * BLK],
                        rows.unsqueeze(1).broadcast_to([4, 32, BLK]),
                    )
                rb_of[sg, mc] = rb

            outq = []

            def mul_unit(sg, mc):
                """multiply for (sg, mc); the output DMA is emitted two
                units later (Sync queue) so its wait-on-mul is pre-satisfied
                and never head-blocks the queue."""
                e_t = e_of.pop((sg, mc))
                rb = rb_of.pop((sg, mc))
                o_t = opool.tile([MC, 2 * SBW], f16, tag="o_t", name="o_t")
                nc.vector.tensor_mul(o_t[:], e_t[:], rb[:])
                outq.append((sg, mc, o_t))
                while len(outq) > 2:
                    emit_out()

            def emit_out():
                sg, mc, o_t = outq.pop(0)
                nc.gpsimd.dma_start(
                    y_out[mc * MC:(mc + 1) * MC,
                          sg * 2 * SBW:(sg + 1) * 2 * SBW],
                    o_t[:],
                )

            pending = []
            fetch_x(0)
            fetch_x(1)
            DRAIN_PER_SLOT = 8
            BCAST_AHEAD = 5

            def drain(n):
                for _ in range(min(n, len(pending))):
                    for k in range(min(BCAST_AHEAD, len(pending))):
                        u = pending[k]
                        if u not in rb_of and u[0] in rec_of:
                            emit_bcast(*u)
                    u = pending.pop(0)
                    if u not in rb_of:
                        emit_bcast(*u)
                    mul_unit(*u)

            for j in range(NSB + 3):
                if j < NSB:
                    fetch_x(j + 2)
                    for mc in range(NCHUNK):
                        mm1_exp(j, mc)
                if j <= NSB and j % 2 == 1:
                    sg = (j - 1) // 2
                    mm2_batch(sg)
                    for q in range(4):
                        emit_recip(sg, q)
                    for mc in range(NCHUNK):
                        pending.append((sg, mc))
                drain(DRAIN_PER_SLOT)
            drain(len(pending))
            assert not pending, len(pending)
            while outq:
                emit_out()

    _split_excess_waits(nc)
    return nc


def _prep_core_inputs(x, normals, offsets):
    f16 = np.float16
    bias = np.einsum("md,md->m", normals, offsets).astype(np.float32)
    w_bf = np.ascontiguousarray(normals.T).astype(f16)

    gid = np.arange(M) // GROUP                     # global group of each m
    gsum = np.zeros((MC, NCHUNK * 32), np.float32)
    for mc in range(NCHUNK):
        for p in range(MC):
            r = gid[mc * MC + p] % 32               # group-within-block row
            gsum[p, mc * 32 + r] = 1.0
    gsum = gsum.astype(f16)
    bias_neg = np.ascontiguousarray(
        -bias.reshape(NCHUNK, MC).T
    ).astype(np.float32)

    in_maps = []
    for b in range(NCORES):
        x_bf = np.ascontiguousarray(x[b].reshape(D, PIX)).astype(f16)
        in_maps.append(
            {"x_bf": x_bf, "normals_bf": w_bf, "gsum_w": gsum,
             "bias_neg": bias_neg}
        )
    return in_maps


def _sibmat_is_expected(sibmat):
    gid = np.arange(M) // GROUP
    expected = (gid[:, None] == gid[None, :]).astype(np.float32)
    return sibmat.shape == (M, M) and np.array_equal(sibmat, expected)


def _numpy_fallback(x, normals, offsets, sibmat):
    # Straight fp32 transcription of the reference; only used if sibmat is
    # not the expected 32-wide block-diagonal matrix.
    bias = np.einsum("md,md->m", normals, offsets)
    out = np.empty((B, M, H, W), np.float32)
    for b in range(B):
        logits = np.einsum("dhw,md->mhw", x[b], normals) - bias[:, None, None]
        logits -= np.max(logits, axis=0, keepdims=True)
        e = np.exp(logits)
        z = np.einsum("mhw,nm->nhw", e, sibmat)
        out[b] = e / (z + 1e-15)
    return out


def kernel(x, normals, offsets, sibmat, steps=None, trace=False, **_ignored):
    from concourse.bass_utils import run_bass_kernel_spmd

    x = np.ascontiguousarray(np.asarray(x, dtype=np.float32))
    normals = np.ascontiguousarray(np.asarray(normals, dtype=np.float32))
    offsets = np.ascontiguousarray(np.asarray(offsets, dtype=np.float32))
    sibmat = np.ascontiguousarray(np.asarray(sibmat, dtype=np.float32))

    if (
        not _sibmat_is_expected(sibmat)
        or np.abs(normals).max() > 0.5
        or np.abs(x).max() > 12.0
    ):
        # unexpected structure or value ranges outside the fp16-safe
        # envelope of the device kernel: compute exactly on host
        return _numpy_fallback(x, normals, offsets, sibmat)

    if "nc" not in _cache:
        _cache["nc"] = _build_nc()
    nc = _cache["nc"]

    in_maps = _prep_core_inputs(x, normals, offsets)
    res = run_bass_kernel_spmd(nc, in_maps, list(range(NCORES)), trace=trace)
    out = np.stack(
        [np.asarray(r["y"]).astype(np.float32).reshape(M, H, W)
         for r in res.results]
    )
    kernel.last_result = res
    return out


# revision 8
# speedup vs baseline: 1.8813x; 1.8753x over previous
"""Trainium2 Bass kernel for hierarchical (sibling-group) softmax over
hyperplane margins.

For x:(8,64,128,128), normals/offsets:(1024,64), sibmat block-diagonal with
32-wide sibling groups:

    logits[b,m,h,w] = <x[b,:,h,w], normals[m]> - <normals[m], offsets[m]>
    out = exp(logits) / (group_sum_32(exp(logits)) + 1e-15)

Sharding: data-parallel over batch, one batch element per NeuronCore (8
cores), no collectives.  Per core, m-chunks of 128 rows live on partitions
and pixels on the free axis.

Design (v2, measured on HW):
  1. PE matmul fp16: logits = normals.T @ x into fp32 PSUM, 4 x N=512 MMs
     per (superblock, chunk) sharing one LDWEIGHTS.
  2. ACT exp PSUM->SBUF fp16 (N=1024 per instruction), per-partition bias
     supplies -<normals_m, offsets_m> in fp32.
  3. PE matmul with per-chunk [128, 32] group indicators accumulates Z for
     4 blocks into one [128, 1024] PSUM tile; the 4 blocks' matmuls use
     col-tile_position 32*bg, so quads execute concurrently in the array
     (~3.7x measured).
  4. DVE reciprocal PSUM->SBUF fp16 (column-quarters to keep the in-order
     DVE stream responsive).
  5. The 1/Z rows are replicated 32x across partitions by SBUF->SBUF DMAs
     whose source AP has a zero-step middle dim (4 rows -> 128 partitions);
     this keeps both operands of the final multiply fp16 in SBUF, so the
     DVE tensor_tensor runs in 2x packed mode (vs 1x reading fp32 PSUM).
  6. DVE multiply e * (1/Z) -> fp16 [128, 2048] tiles, DMAed straight into
     the (M, H, W) output layout; the host upcasts fp16 -> fp32.

All PE work is emitted in long accumulation chains with few semaphore
waits so the HAM clock-gate reaches the warm 2.4 GHz state (cold-start
kernels measure 1.2 GHz).  A post-pass splits multi-wait instructions
(walrus's TRN2 codegen encodes at most one semaphore wait per compute
instruction).  fp16 keeps all value ranges exact to ~1.5e-3 of the fp32
reference (guarded by input-range checks that fall back to an exact host
implementation).
"""

import numpy as np

B, D, H, W = 8, 64, 128, 128
M = 1024
GROUP = 32
PIX = H * W          # 16384 pixels per batch element
BLK = 1024           # pixels per block
SBW = 2048           # pixels per superblock (2 blocks)
NSB = PIX // SBW     # 8 superblocks
MC = 128             # m-chunk width (partition dim)
NCHUNK = M // MC     # 8
NCORES = 8
FMAX = 512           # max moving free dim into one PSUM bank (fp32 out)
NSG = 4              # supergroups (4 blocks each) per core

_cache = {}


_WAIT_OK_OPCODES = {"Call"}


def _split_excess_waits(nc):
    """Walrus's TRN2 codegen (CoreV3GenImpl setupSyncWait) encodes at most
    one semaphore wait per compute instruction (Matmult, TensorTensor, ...);
    Tile can legitimately attach several (e.g. waits on two input DMAs).
    Move all but one wait onto EventSemaphore instructions inserted just
    before the instruction on the same engine — ordering is identical."""
    import concourse.mybir as mybir

    n_fixed = 0
    for f in nc.m.functions:
        for blk in f.blocks:
            out = []
            changed = False
            for inst in blk.instructions:
                si = inst.sync_info
                if (
                    si is not None
                    and len(si.on_wait) > 1
                    and inst.opcode not in _WAIT_OK_OPCODES
                ):
                    waits = list(si.on_wait)
                    for j, w in enumerate(waits[:-1]):
                        out.append(
                            mybir.InstEventSemaphore(
                                name=f"{inst.name}-wsplit{j}",
                                opcode="EventSemaphore",
                                engine=inst.engine,
                                sync_info=mybir.SyncInfo(
                                    on_wait=[w], on_update=[]
                                ),
                            )
                        )
                    inst.sync_info = mybir.SyncInfo(
                        on_wait=[waits[-1]], on_update=list(si.on_update)
                    )
                    changed = True
                    n_fixed += 1
                out.append(inst)
            if changed:
                blk.instructions = out
    return n_fixed


def _build_nc():
    import concourse.bass as bass
    import concourse.mybir as mybir
    import concourse.tile as tile

    f32 = mybir.dt.float32
    f16 = mybir.dt.float16
    nc = bass.Bass()

    x_in = nc.declare_dram_parameter("x_bf", [D, PIX], f16, isOutput=False)
    w_in = nc.declare_dram_parameter("normals_bf", [D, M], f16, isOutput=False)
    # gsum_w[:, mc*32:(mc+1)*32]: [128, 32] indicator, [p, r] = 1 iff
    # r == 4*mc + p//32 — maps chunk mc's partitions to its 32 global
    # groups-within-block rows of the Z tile.
    g_in = nc.declare_dram_parameter("gsum_w", [MC, NCHUNK * 32], f16,
                                     isOutput=False)
    # bias_neg[p, mc] = -<normals, offsets> for m = mc*128+p; applied as the
    # ACT exp per-partition bias (exact fp32).
    # gbc_w[32*q + r, mc*128 + p]: replica q of the [32, 128] indicator that
    # broadcasts group row 4*mc + p//32 onto chunk partitions; replicas let
    # mm3's stationary start at the same partition base as its moving
    # operand (rows 32*bgl of the rec tile).
    b_in = nc.declare_dram_parameter("gbc_w", [4 * 32, NCHUNK * MC], f16,
                                     isOutput=False)
    c_in = nc.declare_dram_parameter("bias_neg", [MC, NCHUNK], f32,
                                     isOutput=False)
    y_out = nc.declare_dram_parameter("y", [M, PIX], f16, isOutput=True)

    with tile.TileContext(nc) as tc:
        with (
            tc.tile_pool(name="const", bufs=1) as cpool,
            tc.tile_pool(name="xin", bufs=3) as xpool,
            tc.tile_pool(name="expv", bufs=16) as epool,
            tc.tile_pool(name="recv", bufs=2) as rpool,
            tc.tile_pool(name="outv", bufs=4) as opool,
            tc.tile_pool(name="psl", bufs=2, space="PSUM") as pslp,
            tc.tile_pool(name="psz", bufs=1, space="PSUM") as pszp,
            tc.tile_pool(name="psb", bufs=2, space="PSUM") as psbp,
        ):
            w_sb = cpool.tile([D, M], f16)
            nc.sync.dma_start(w_sb[:], w_in[:])
            g_sb = cpool.tile([MC, NCHUNK * 32], f16)
            nc.sync.dma_start(g_sb[:], g_in[:])
            b_sb = cpool.tile([4 * 32, NCHUNK * MC], f16)
            nc.sync.dma_start(b_sb[:], b_in[:])
            c_sb = cpool.tile([MC, NCHUNK], f32)
            nc.sync.dma_start(c_sb[:], c_in[:])

            x_of = {}
            e_of = {}
            rec_of = {}
            psz_of = {}

            def fetch_x(sb):
                if sb in x_of or sb >= NSB:
                    return
                x_t = xpool.tile([D, SBW], f16, tag="x_t", name="x_t")
                nc.sync.dma_start(x_t[:], x_in[:, sb * SBW:(sb + 1) * SBW])
                x_of[sb] = x_t

            def mm1_exp(sb, mc):
                """logits + exp for chunk mc of superblock sb (2 blocks).
                e tiles span a whole supergroup [128, 4096] so the tail
                (broadcast/multiply/out-DMA) works in coarse units."""
                x_t = x_of[sb]
                sg = sb // 2
                if sb % 2 == 0:
                    e_of[sg, mc] = epool.tile([MC, 2 * SBW], f16,
                                              tag="e_t", name="e_t")
                e_t = e_of[sg, mc]
                for b in range(2):          # block within superblock
                    cb = (sb % 2) * SBW + b * BLK
                    ps = pslp.tile([MC, BLK], f32, tag="ps_l", name="ps_l")
                    for h in range(2):
                        nc.tensor.matmul(
                            ps[:, h * FMAX:(h + 1) * FMAX],
                            w_sb[:, mc * MC:(mc + 1) * MC],
                            x_t[:, b * BLK + h * FMAX:
                                b * BLK + (h + 1) * FMAX],
                            start=True, stop=True,
                        )
                    nc.scalar.activation(
                        e_t[:, cb:cb + BLK], ps[:],
                        mybir.ActivationFunctionType.Exp,
                        bias=c_sb[:, mc:mc + 1],
                    )

            def mm2_batch(sg):
                """Z for supergroup sg (blocks 4sg..4sg+3) into one
                [128, 1024] PSUM tile; 4 blocks via col-tile_position run
                concurrently in the PE array."""
                ps_z = pszp.tile([4 * 32, BLK], f32, tag="ps_z", name="ps_z")
                psz_of[sg] = ps_z
                for half in range(2):
                    for mc in range(NCHUNK):
                        for bgl in range(4):
                            e_t = e_of[sg, mc]
                            nc.tensor.matmul(
                                ps_z[32 * bgl:32 * (bgl + 1),
                                     half * FMAX:(half + 1) * FMAX],
                                g_sb[:, mc * 32:(mc + 1) * 32],
                                e_t[:, bgl * BLK + half * FMAX:
                                    bgl * BLK + (half + 1) * FMAX],
                                start=(mc == 0), stop=(mc == NCHUNK - 1),
                                tile_position=(0, 32 * bgl),
                                skip_group_check=True,
                            )

            def emit_recip(sg, q):
                QW = BLK // 4
                if q == 0:
                    rec_of[sg] = rpool.tile([4 * 32, BLK], f16, tag="rec",
                                            name="rec")
                rec = rec_of[sg]
                ps_z = psz_of[sg]
                with nc.allow_low_precision(
                    reason="fp16 rounding of reciprocal feeding the "
                    "broadcast; well within output tolerance"
                ):
                    nc.vector.reciprocal(
                        rec[:, q * QW:(q + 1) * QW],
                        ps_z[:, q * QW:(q + 1) * QW],
                    )
                if q == 3:
                    del psz_of[sg]

            outq = []

            def mul_unit(sg, mc):
                """PE broadcasts 1/Z rows onto the chunk's 128 partitions
                (PSUM, 4-way row-packed across the 4 blocks), then DVE
                multiplies e * (1/Z) at 1x from PSUM.  The output DMA is
                emitted two units later so its wait-on-mul is pre-satisfied
                and never head-blocks the Sync queue."""
                rec = rec_of[sg]
                e_t = e_of.pop((sg, mc))
                o_t = opool.tile([MC, 2 * SBW], f16, tag="o_t", name="o_t")
                for h in range(2):
                    for bgl in range(4):
                        ps_b = psbp.tile([MC, FMAX], f32, tag="ps_b",
                                         name="ps_b")
                        nc.tensor.matmul(
                            ps_b[:],
                            b_sb[32 * bgl:32 * (bgl + 1),
                                 mc * MC:(mc + 1) * MC],
                            rec[32 * bgl:32 * (bgl + 1),
                                h * FMAX:(h + 1) * FMAX],
                            start=True, stop=True,
                            tile_position=(32 * bgl, 0),
                        )
                        seg = bgl * BLK + h * FMAX
                        nc.vector.tensor_mul(
                            o_t[:, seg:seg + FMAX],
                            e_t[:, seg:seg + FMAX],
                            ps_b[:],
                        )
                outq.append((sg, mc, o_t))
                while len(outq) > 2:
                    emit_out()

            def emit_out():
                sg, mc, o_t = outq.pop(0)
                nc.sync.dma_start(
                    y_out[mc * MC:(mc + 1) * MC,
                          sg * 2 * SBW:(sg + 1) * 2 * SBW],
                    o_t[:],
                )

            pending = []
            fetch_x(0)
            fetch_x(1)
            DRAIN_PER_SLOT = 8

            def drain(n):
                for _ in range(min(n, len(pending))):
                    mul_unit(*pending.pop(0))

            for j in range(NSB + 3):
                if j < NSB:
                    fetch_x(j + 2)
                    for mc in range(NCHUNK):
                        mm1_exp(j, mc)
                if j <= NSB and j % 2 == 1:
                    sg = (j - 1) // 2
                    mm2_batch(sg)
                    for q in range(4):
                        emit_recip(sg, q)
                    for mc in range(NCHUNK):
                        pending.append((sg, mc))
                drain(DRAIN_PER_SLOT)
            drain(len(pending))
            assert not pending, len(pending)
            while outq:
                emit_out()

    _split_excess_waits(nc)
    return nc


def _prep_core_inputs(x, normals, offsets):
    f16 = np.float16
    bias = np.einsum("md,md->m", normals, offsets).astype(np.float32)
    w_bf = np.ascontiguousarray(normals.T).astype(f16)

    gid = np.arange(M) // GROUP                     # global group of each m
    gsum = np.zeros((MC, NCHUNK * 32), np.float32)
    for mc in range(NCHUNK):
        for p in range(MC):
            r = gid[mc * MC + p] % 32               # group-within-block row
            gsum[p, mc * 32 + r] = 1.0
    gsum = gsum.astype(f16)
    gbc = np.zeros((32, NCHUNK * MC), np.float32)
    for mc in range(NCHUNK):
        for p in range(MC):
            r = gid[mc * MC + p] % 32
            gbc[r, mc * MC + p] = 1.0
    gbc = np.tile(gbc, (4, 1)).astype(f16)
    bias_neg = np.ascontiguousarray(
        -bias.reshape(NCHUNK, MC).T
    ).astype(np.float32)

    in_maps = []
    for b in range(NCORES):
        x_bf = np.ascontiguousarray(x[b].reshape(D, PIX)).astype(f16)
        in_maps.append(
            {"x_bf": x_bf, "normals_bf": w_bf, "gsum_w": gsum,
             "gbc_w": gbc, "bias_neg": bias_neg}
        )
    return in_maps


def _sibmat_is_expected(sibmat):
    gid = np.arange(M) // GROUP
    expected = (gid[:, None] == gid[None, :]).astype(np.float32)
    return sibmat.shape == (M, M) and np.array_equal(sibmat, expected)


def _numpy_fallback(x, normals, offsets, sibmat):
    # Straight fp32 transcription of the reference; only used if sibmat is
    # not the expected 32-wide block-diagonal matrix.
    bias = np.einsum("md,md->m", normals, offsets)
    out = np.empty((B, M, H, W), np.float32)
    for b in range(B):
        logits = np.einsum("dhw,md->mhw", x[b], normals) - bias[:, None, None]
        logits -= np.max(logits, axis=0, keepdims=True)
        e = np.exp(logits)
        z = np.einsum("mhw,nm->nhw", e, sibmat)
        out[b] = e / (z + 1e-15)
    return out


def kernel(x, normals, offsets, sibmat, steps=None, trace=False, **_ignored):
    from concourse.bass_utils import run_bass_kernel_spmd

    x = np.ascontiguousarray(np.asarray(x, dtype=np.float32))
    normals = np.ascontiguousarray(np.asarray(normals, dtype=np.float32))
    offsets = np.ascontiguousarray(np.asarray(offsets, dtype=np.float32))
    sibmat = np.ascontiguousarray(np.asarray(sibmat, dtype=np.float32))

    if (
        not _sibmat_is_expected(sibmat)
        or np.abs(normals).max() > 0.5
        or np.abs(x).max() > 12.0
    ):
        # unexpected structure or value ranges outside the fp16-safe
        # envelope of the device kernel: compute exactly on host
        return _numpy_fallback(x, normals, offsets, sibmat)

    if "nc" not in _cache:
        _cache["nc"] = _build_nc()
    nc = _cache["nc"]

    in_maps = _prep_core_inputs(x, normals, offsets)
    res = run_bass_kernel_spmd(nc, in_maps, list(range(NCORES)), trace=trace)
    out = np.stack(
        [np.asarray(r["y"]).astype(np.float32).reshape(M, H, W)
         for r in res.results]
    )
    kernel.last_result = res
    return out


# revision 9
# speedup vs baseline: 1.9561x; 1.0398x over previous
"""Trainium2 Bass kernel for hierarchical (sibling-group) softmax over
hyperplane margins.

For x:(8,64,128,128), normals/offsets:(1024,64), sibmat block-diagonal with
32-wide sibling groups:

    logits[b,m,h,w] = <x[b,:,h,w], normals[m]> - <normals[m], offsets[m]>
    out = exp(logits) / (group_sum_32(exp(logits)) + 1e-15)

Sharding: data-parallel over batch, one batch element per NeuronCore (8
cores), no collectives.  Per core, m-chunks of 128 rows live on partitions
and pixels on the free axis.

Design (v2, measured on HW):
  1. PE matmul fp16: logits = normals.T @ x into fp32 PSUM, 4 x N=512 MMs
     per (superblock, chunk) sharing one LDWEIGHTS.
  2. ACT exp PSUM->SBUF fp16 (N=1024 per instruction), per-partition bias
     supplies -<normals_m, offsets_m> in fp32.
  3. PE matmul with per-chunk [128, 32] group indicators accumulates Z for
     4 blocks into one [128, 1024] PSUM tile; the 4 blocks' matmuls use
     col-tile_position 32*bg, so quads execute concurrently in the array
     (~3.7x measured).
  4. DVE reciprocal PSUM->SBUF fp16 (column-quarters to keep the in-order
     DVE stream responsive).
  5. The 1/Z rows are replicated 32x across partitions by SBUF->SBUF DMAs
     whose source AP has a zero-step middle dim (4 rows -> 128 partitions);
     this keeps both operands of the final multiply fp16 in SBUF, so the
     DVE tensor_tensor runs in 2x packed mode (vs 1x reading fp32 PSUM).
  6. DVE multiply e * (1/Z) -> fp16 [128, 2048] tiles, DMAed straight into
     the (M, H, W) output layout; the host upcasts fp16 -> fp32.

All PE work is emitted in long accumulation chains with few semaphore
waits so the HAM clock-gate reaches the warm 2.4 GHz state (cold-start
kernels measure 1.2 GHz).  A post-pass splits multi-wait instructions
(walrus's TRN2 codegen encodes at most one semaphore wait per compute
instruction).  fp16 keeps all value ranges exact to ~1.5e-3 of the fp32
reference (guarded by input-range checks that fall back to an exact host
implementation).
"""

import numpy as np

B, D, H, W = 8, 64, 128, 128
M = 1024
GROUP = 32
PIX = H * W          # 16384 pixels per batch element
BLK = 1024           # pixels per block
SBW = 2048           # pixels per superblock (2 blocks)
NSB = PIX // SBW     # 8 superblocks
MC = 128             # m-chunk width (partition dim)
NCHUNK = M // MC     # 8
NCORES = 8
FMAX = 512           # max moving free dim into one PSUM bank (fp32 out)
NSG = 4              # supergroups (4 blocks each) per core

_cache = {}


_WAIT_OK_OPCODES = {"Call"}


def _split_excess_waits(nc):
    """Walrus's TRN2 codegen (CoreV3GenImpl setupSyncWait) encodes at most
    one semaphore wait per compute instruction (Matmult, TensorTensor, ...);
    Tile can legitimately attach several (e.g. waits on two input DMAs).
    Move all but one wait onto EventSemaphore instructions inserted just
    before the instruction on the same engine — ordering is identical."""
    import concourse.mybir as mybir

    n_fixed = 0
    for f in nc.m.functions:
        for blk in f.blocks:
            out = []
            changed = False
            for inst in blk.instructions:
                si = inst.sync_info
                if (
                    si is not None
                    and len(si.on_wait) > 1
                    and inst.opcode not in _WAIT_OK_OPCODES
                ):
                    waits = list(si.on_wait)
                    for j, w in enumerate(waits[:-1]):
                        out.append(
                            mybir.InstEventSemaphore(
                                name=f"{inst.name}-wsplit{j}",
                                opcode="EventSemaphore",
                                engine=inst.engine,
                                sync_info=mybir.SyncInfo(
                                    on_wait=[w], on_update=[]
                                ),
                            )
                        )
                    inst.sync_info = mybir.SyncInfo(
                        on_wait=[waits[-1]], on_update=list(si.on_update)
                    )
                    changed = True
                    n_fixed += 1
                out.append(inst)
            if changed:
                blk.instructions = out
    return n_fixed


def _build_nc():
    import concourse.bass as bass
    import concourse.mybir as mybir
    import concourse.tile as tile

    f32 = mybir.dt.float32
    f16 = mybir.dt.float16
    nc = bass.Bass()

    x_in = nc.declare_dram_parameter("x_bf", [D, PIX], f16, isOutput=False)
    w_in = nc.declare_dram_parameter("normals_bf", [D, M], f16, isOutput=False)
    # gsum_w[:, mc*32:(mc+1)*32]: [128, 32] indicator, [p, r] = 1 iff
    # r == 4*mc + p//32 — maps chunk mc's partitions to its 32 global
    # groups-within-block rows of the Z tile.
    g_in = nc.declare_dram_parameter("gsum_w", [MC, NCHUNK * 32], f16,
                                     isOutput=False)
    # bias_neg[p, mc] = -<normals, offsets> for m = mc*128+p; applied as the
    # ACT exp per-partition bias (exact fp32).
    # gbc_w[32*q + r, mc*128 + p]: replica q of the [32, 128] indicator that
    # broadcasts group row 4*mc + p//32 onto chunk partitions; replicas let
    # mm3's stationary start at the same partition base as its moving
    # operand (rows 32*bgl of the rec tile).
    b_in = nc.declare_dram_parameter("gbc_w", [4 * 32, NCHUNK * MC], f16,
                                     isOutput=False)
    c_in = nc.declare_dram_parameter("bias_neg", [MC, NCHUNK], f32,
                                     isOutput=False)
    y_out = nc.declare_dram_parameter("y", [M, PIX], f16, isOutput=True)

    with tile.TileContext(nc) as tc:
        with (
            tc.tile_pool(name="const", bufs=1) as cpool,
            tc.tile_pool(name="xin", bufs=3) as xpool,
            tc.tile_pool(name="expv", bufs=16) as epool,
            tc.tile_pool(name="recv", bufs=2) as rpool,
            tc.tile_pool(name="lnzv", bufs=2) as lpool,
            tc.tile_pool(name="outv", bufs=4) as opool,
            tc.tile_pool(name="psl", bufs=2, space="PSUM") as pslp,
            tc.tile_pool(name="psz", bufs=1, space="PSUM") as pszp,
            tc.tile_pool(name="psb", bufs=2, space="PSUM") as psbp,
        ):
            w_sb = cpool.tile([D, M], f16)
            nc.sync.dma_start(w_sb[:], w_in[:])
            g_sb = cpool.tile([MC, NCHUNK * 32], f16)
            nc.sync.dma_start(g_sb[:], g_in[:])
            b_sb = cpool.tile([4 * 32, NCHUNK * MC], f16)
            nc.sync.dma_start(b_sb[:], b_in[:])
            c_sb = cpool.tile([MC, NCHUNK], f32)
            nc.sync.dma_start(c_sb[:], c_in[:])

            x_of = {}
            e_of = {}
            rec_of = {}
            psz_of = {}

            def fetch_x(sb):
                if sb in x_of or sb >= NSB:
                    return
                x_t = xpool.tile([D, SBW], f16, tag="x_t", name="x_t")
                nc.sync.dma_start(x_t[:], x_in[:, sb * SBW:(sb + 1) * SBW])
                x_of[sb] = x_t

            def mm1_exp(sb, mc):
                """logits + exp for chunk mc of superblock sb (2 blocks).
                e tiles span a whole supergroup [128, 4096] so the tail
                (broadcast/multiply/out-DMA) works in coarse units."""
                x_t = x_of[sb]
                sg = sb // 2
                if sb % 2 == 0:
                    e_of[sg, mc] = epool.tile([MC, 2 * SBW], f16,
                                              tag="e_t", name="e_t")
                e_t = e_of[sg, mc]
                for b in range(2):          # block within superblock
                    cb = (sb % 2) * SBW + b * BLK
                    ps = pslp.tile([MC, BLK], f32, tag="ps_l", name="ps_l")
                    for h in range(2):
                        nc.tensor.matmul(
                            ps[:, h * FMAX:(h + 1) * FMAX],
                            w_sb[:, mc * MC:(mc + 1) * MC],
                            x_t[:, b * BLK + h * FMAX:
                                b * BLK + (h + 1) * FMAX],
                            start=True, stop=True,
                        )
                    nc.scalar.activation(
                        e_t[:, cb:cb + BLK], ps[:],
                        mybir.ActivationFunctionType.Exp,
                        bias=c_sb[:, mc:mc + 1],
                    )

            def mm2_batch(sg):
                """Z for supergroup sg (blocks 4sg..4sg+3) into one
                [128, 1024] PSUM tile; 4 blocks via col-tile_position run
                concurrently in the PE array."""
                ps_z = pszp.tile([4 * 32, BLK], f32, tag="ps_z", name="ps_z")
                psz_of[sg] = ps_z
                for half in range(2):
                    for mc in range(NCHUNK):
                        for bgl in range(4):
                            e_t = e_of[sg, mc]
                            nc.tensor.matmul(
                                ps_z[32 * bgl:32 * (bgl + 1),
                                     half * FMAX:(half + 1) * FMAX],
                                g_sb[:, mc * 32:(mc + 1) * 32],
                                e_t[:, bgl * BLK + half * FMAX:
                                    bgl * BLK + (half + 1) * FMAX],
                                start=(mc == 0), stop=(mc == NCHUNK - 1),
                                tile_position=(0, 32 * bgl),
                                skip_group_check=True,
                            )

            def emit_recip(sg, q):
                # 1/Z = exp(-ln Z) on ACT (Ln and Exp share one table set);
                # the DVE's iterative-divide reciprocal costs ~6.3 cyc/elem
                # and the DVE queue is the critical path.
                QW = BLK // 2
                if q == 0:
                    rec_of[sg] = rpool.tile([4 * 32, BLK], f16, tag="rec",
                                            name="rec")
                rec = rec_of[sg]
                ps_z = psz_of[sg]
                lnz = lpool.tile([4 * 32, QW], f32, tag="lnz", name="lnz")
                nc.scalar.activation(
                    lnz[:], ps_z[:, q * QW:(q + 1) * QW],
                    mybir.ActivationFunctionType.Ln,
                )
                with nc.allow_low_precision(
                    reason="fp16 rounding of 1/Z feeding the broadcast "
                    "matmul; well within output tolerance"
                ):
                    nc.scalar.activation(
                        rec[:, q * QW:(q + 1) * QW], lnz[:],
                        mybir.ActivationFunctionType.Exp,
                        scale=-1.0,
                    )
                if q == 1:
                    del psz_of[sg]

            outq = []

            def mul_unit(sg, mc):
                """PE broadcasts 1/Z rows onto the chunk's 128 partitions
                (PSUM, 4-way row-packed across the 4 blocks), then DVE
                multiplies e * (1/Z) at 1x from PSUM.  The output DMA is
                emitted two units later so its wait-on-mul is pre-satisfied
                and never head-blocks the Sync queue."""
                rec = rec_of[sg]
                e_t = e_of.pop((sg, mc))
                o_t = opool.tile([MC, 2 * SBW], f16, tag="o_t", name="o_t")
                for h in range(2):
                    for bgl in range(4):
                        ps_b = psbp.tile([MC, FMAX], f32, tag="ps_b",
                                         name="ps_b")
                        nc.tensor.matmul(
                            ps_b[:],
                            b_sb[32 * bgl:32 * (bgl + 1),
                                 mc * MC:(mc + 1) * MC],
                            rec[32 * bgl:32 * (bgl + 1),
                                h * FMAX:(h + 1) * FMAX],
                            start=True, stop=True,
                            tile_position=(32 * bgl, 0),
                        )
                        seg = bgl * BLK + h * FMAX
                        nc.vector.tensor_mul(
                            o_t[:, seg:seg + FMAX],
                            e_t[:, seg:seg + FMAX],
                            ps_b[:],
                        )
                outq.append((sg, mc, o_t))
                while len(outq) > 2:
                    emit_out()

            def emit_out():
                sg, mc, o_t = outq.pop(0)
                nc.sync.dma_start(
                    y_out[mc * MC:(mc + 1) * MC,
                          sg * 2 * SBW:(sg + 1) * 2 * SBW],
                    o_t[:],
                )

            pending = []
            fetch_x(0)
            fetch_x(1)
            DRAIN_PER_SLOT = 8

            def drain(n):
                for _ in range(min(n, len(pending))):
                    mul_unit(*pending.pop(0))

            for j in range(NSB + 3):
                if j < NSB:
                    fetch_x(j + 2)
                    for mc in range(NCHUNK):
                        mm1_exp(j, mc)
                if j <= NSB and j % 2 == 1:
                    sg = (j - 1) // 2
                    mm2_batch(sg)
                    for q in range(2):
                        emit_recip(sg, q)
                    for mc in range(NCHUNK):
                        pending.append((sg, mc))
                drain(DRAIN_PER_SLOT)
            drain(len(pending))
            assert not pending, len(pending)
            while outq:
                emit_out()

    _split_excess_waits(nc)
    return nc


def _prep_core_inputs(x, normals, offsets):
    f16 = np.float16
    bias = np.einsum("md,md->m", normals, offsets).astype(np.float32)
    w_bf = np.ascontiguousarray(normals.T).astype(f16)

    gid = np.arange(M) // GROUP                     # global group of each m
    gsum = np.zeros((MC, NCHUNK * 32), np.float32)
    for mc in range(NCHUNK):
        for p in range(MC):
            r = gid[mc * MC + p] % 32               # group-within-block row
            gsum[p, mc * 32 + r] = 1.0
    gsum = gsum.astype(f16)
    gbc = np.zeros((32, NCHUNK * MC), np.float32)
    for mc in range(NCHUNK):
        for p in range(MC):
            r = gid[mc * MC + p] % 32
            gbc[r, mc * MC + p] = 1.0
    gbc = np.tile(gbc, (4, 1)).astype(f16)
    bias_neg = np.ascontiguousarray(
        -bias.reshape(NCHUNK, MC).T
    ).astype(np.float32)

    in_maps = []
    for b in range(NCORES):
        x_bf = np.ascontiguousarray(x[b].reshape(D, PIX)).astype(f16)
        in_maps.append(
            {"x_bf": x_bf, "normals_bf": w_bf, "gsum_w": gsum,
             "gbc_w": gbc, "bias_neg": bias_neg}
        )
    return in_maps


def _sibmat_is_expected(sibmat):
    gid = np.arange(M) // GROUP
    expected = (gid[:, None] == gid[None, :]).astype(np.float32)
    return sibmat.shape == (M, M) and np.array_equal(sibmat, expected)


def _numpy_fallback(x, normals, offsets, sibmat):
    # Straight fp32 transcription of the reference; only used if sibmat is
    # not the expected 32-wide block-diagonal matrix.
    bias = np.einsum("md,md->m", normals, offsets)
    out = np.empty((B, M, H, W), np.float32)
    for b in range(B):
        logits = np.einsum("dhw,md->mhw", x[b], normals) - bias[:, None, None]
        logits -= np.max(logits, axis=0, keepdims=True)
        e = np.exp(logits)
        z = np.einsum("mhw,nm->nhw", e, sibmat)
        out[b] = e / (z + 1e-15)
    return out


def kernel(x, normals, offsets, sibmat, steps=None, trace=False, **_ignored):
    from concourse.bass_utils import run_bass_kernel_spmd

    x = np.ascontiguousarray(np.asarray(x, dtype=np.float32))
    normals = np.ascontiguousarray(np.asarray(normals, dtype=np.float32))
    offsets = np.ascontiguousarray(np.asarray(offsets, dtype=np.float32))
    sibmat = np.ascontiguousarray(np.asarray(sibmat, dtype=np.float32))

    if (
        not _sibmat_is_expected(sibmat)
        or np.abs(normals).max() > 0.5
        or np.abs(x).max() > 12.0
    ):
        # unexpected structure or value ranges outside the fp16-safe
        # envelope of the device kernel: compute exactly on host
        return _numpy_fallback(x, normals, offsets, sibmat)

    if "nc" not in _cache:
        _cache["nc"] = _build_nc()
    nc = _cache["nc"]

    in_maps = _prep_core_inputs(x, normals, offsets)
    res = run_bass_kernel_spmd(nc, in_maps, list(range(NCORES)), trace=trace)
    out = np.stack(
        [np.asarray(r["y"]).astype(np.float32).reshape(M, H, W)
         for r in res.results]
    )
    kernel.last_result = res
    return out


# revision 10
# speedup vs baseline: 1.9624x; 1.0032x over previous
"""Trainium2 Bass kernel for hierarchical (sibling-group) softmax over
hyperplane margins.

For x:(8,64,128,128), normals/offsets:(1024,64), sibmat block-diagonal with
32-wide sibling groups:

    logits[b,m,h,w] = <x[b,:,h,w], normals[m]> - <normals[m], offsets[m]>
    out = exp(logits) / (group_sum_32(exp(logits)) + 1e-15)

Sharding: data-parallel over batch, one batch element per NeuronCore (8
cores), no collectives.  Per core, m-chunks of 128 rows live on partitions
and pixels on the free axis.

Design (v2, measured on HW):
  1. PE matmul fp16: logits = normals.T @ x into fp32 PSUM, 4 x N=512 MMs
     per (superblock, chunk) sharing one LDWEIGHTS.
  2. ACT exp PSUM->SBUF fp16 (N=1024 per instruction), per-partition bias
     supplies -<normals_m, offsets_m> in fp32.
  3. PE matmul with per-chunk [128, 32] group indicators accumulates Z for
     4 blocks into one [128, 1024] PSUM tile; the 4 blocks' matmuls use
     col-tile_position 32*bg, so quads execute concurrently in the array
     (~3.7x measured).
  4. DVE reciprocal PSUM->SBUF fp16 (column-quarters to keep the in-order
     DVE stream responsive).
  5. The 1/Z rows are replicated 32x across partitions by SBUF->SBUF DMAs
     whose source AP has a zero-step middle dim (4 rows -> 128 partitions);
     this keeps both operands of the final multiply fp16 in SBUF, so the
     DVE tensor_tensor runs in 2x packed mode (vs 1x reading fp32 PSUM).
  6. DVE multiply e * (1/Z) -> fp16 [128, 2048] tiles, DMAed straight into
     the (M, H, W) output layout; the host upcasts fp16 -> fp32.

All PE work is emitted in long accumulation chains with few semaphore
waits so the HAM clock-gate reaches the warm 2.4 GHz state (cold-start
kernels measure 1.2 GHz).  A post-pass splits multi-wait instructions
(walrus's TRN2 codegen encodes at most one semaphore wait per compute
instruction).  fp16 keeps all value ranges exact to ~1.5e-3 of the fp32
reference (guarded by input-range checks that fall back to an exact host
implementation).
"""

import numpy as np

B, D, H, W = 8, 64, 128, 128
M = 1024
GROUP = 32
PIX = H * W          # 16384 pixels per batch element
BLK = 1024           # pixels per block
SBW = 2048           # pixels per superblock (2 blocks)
NSB = PIX // SBW     # 8 superblocks
MC = 128             # m-chunk width (partition dim)
NCHUNK = M // MC     # 8
NCORES = 8
FMAX = 512           # max moving free dim into one PSUM bank (fp32 out)
NSG = 4              # supergroups (4 blocks each) per core

_cache = {}


_WAIT_OK_OPCODES = {"Call"}


def _split_excess_waits(nc):
    """Walrus's TRN2 codegen (CoreV3GenImpl setupSyncWait) encodes at most
    one semaphore wait per compute instruction (Matmult, TensorTensor, ...);
    Tile can legitimately attach several (e.g. waits on two input DMAs).
    Move all but one wait onto EventSemaphore instructions inserted just
    before the instruction on the same engine — ordering is identical."""
    import concourse.mybir as mybir

    n_fixed = 0
    for f in nc.m.functions:
        for blk in f.blocks:
            out = []
            changed = False
            for inst in blk.instructions:
                si = inst.sync_info
                if (
                    si is not None
                    and len(si.on_wait) > 1
                    and inst.opcode not in _WAIT_OK_OPCODES
                ):
                    waits = list(si.on_wait)
                    for j, w in enumerate(waits[:-1]):
                        out.append(
                            mybir.InstEventSemaphore(
                                name=f"{inst.name}-wsplit{j}",
                                opcode="EventSemaphore",
                                engine=inst.engine,
                                sync_info=mybir.SyncInfo(
                                    on_wait=[w], on_update=[]
                                ),
                            )
                        )
                    inst.sync_info = mybir.SyncInfo(
                        on_wait=[waits[-1]], on_update=list(si.on_update)
                    )
                    changed = True
                    n_fixed += 1
                out.append(inst)
            if changed:
                blk.instructions = out
    return n_fixed


def _build_nc():
    import concourse.bass as bass
    import concourse.mybir as mybir
    import concourse.tile as tile

    f32 = mybir.dt.float32
    f16 = mybir.dt.float16
    nc = bass.Bass()

    x_in = nc.declare_dram_parameter("x_bf", [D, PIX], f16, isOutput=False)
    w_in = nc.declare_dram_parameter("normals_bf", [D, M], f16, isOutput=False)
    # gsum_w[:, mc*32:(mc+1)*32]: [128, 32] indicator, [p, r] = 1 iff
    # r == 4*mc + p//32 — maps chunk mc's partitions to its 32 global
    # groups-within-block rows of the Z tile.
    g_in = nc.declare_dram_parameter("gsum_w", [MC, NCHUNK * 32], f16,
                                     isOutput=False)
    # bias_neg[p, mc] = -<normals, offsets> for m = mc*128+p; applied as the
    # ACT exp per-partition bias (exact fp32).
    # gbc_w[32*q + r, mc*128 + p]: replica q of the [32, 128] indicator that
    # broadcasts group row 4*mc + p//32 onto chunk partitions; replicas let
    # mm3's stationary start at the same partition base as its moving
    # operand (rows 32*bgl of the rec tile).
    b_in = nc.declare_dram_parameter("gbc_w", [4 * 32, NCHUNK * MC], f16,
                                     isOutput=False)
    c_in = nc.declare_dram_parameter("bias_neg", [MC, NCHUNK], f32,
                                     isOutput=False)
    y_out = nc.declare_dram_parameter("y", [M, PIX], f16, isOutput=True)

    with tile.TileContext(nc) as tc:
        with (
            tc.tile_pool(name="const", bufs=1) as cpool,
            tc.tile_pool(name="xin", bufs=3) as xpool,
            tc.tile_pool(name="expv", bufs=16) as epool,
            tc.tile_pool(name="recv", bufs=2) as rpool,
            tc.tile_pool(name="lnzv", bufs=2) as lpool,
            tc.tile_pool(name="outv", bufs=4) as opool,
            tc.tile_pool(name="psl", bufs=2, space="PSUM") as pslp,
            tc.tile_pool(name="psz", bufs=1, space="PSUM") as pszp,
            tc.tile_pool(name="psb", bufs=2, space="PSUM") as psbp,
        ):
            w_sb = cpool.tile([D, M], f16)
            nc.sync.dma_start(w_sb[:], w_in[:])
            g_sb = cpool.tile([MC, NCHUNK * 32], f16)
            nc.sync.dma_start(g_sb[:], g_in[:])
            b_sb = cpool.tile([4 * 32, NCHUNK * MC], f16)
            nc.sync.dma_start(b_sb[:], b_in[:])
            c_sb = cpool.tile([MC, NCHUNK], f32)
            nc.sync.dma_start(c_sb[:], c_in[:])

            x_of = {}
            e_of = {}
            rec_of = {}
            psz_of = {}

            def fetch_x(sb):
                if sb in x_of or sb >= NSB:
                    return
                x_t = xpool.tile([D, SBW], f16, tag="x_t", name="x_t")
                nc.sync.dma_start(x_t[:], x_in[:, sb * SBW:(sb + 1) * SBW])
                x_of[sb] = x_t

            def mm1_exp(sb, mc):
                """logits + exp for chunk mc of superblock sb (2 blocks).
                e tiles span a whole supergroup [128, 4096] so the tail
                (broadcast/multiply/out-DMA) works in coarse units."""
                x_t = x_of[sb]
                sg = sb // 2
                if sb % 2 == 0:
                    e_of[sg, mc] = epool.tile([MC, 2 * SBW], f16,
                                              tag="e_t", name="e_t")
                e_t = e_of[sg, mc]
                for b in range(2):          # block within superblock
                    cb = (sb % 2) * SBW + b * BLK
                    ps = pslp.tile([MC, BLK], f32, tag="ps_l", name="ps_l")
                    for h in range(2):
                        nc.tensor.matmul(
                            ps[:, h * FMAX:(h + 1) * FMAX],
                            w_sb[:, mc * MC:(mc + 1) * MC],
                            x_t[:, b * BLK + h * FMAX:
                                b * BLK + (h + 1) * FMAX],
                            start=True, stop=True,
                        )
                    nc.scalar.activation(
                        e_t[:, cb:cb + BLK], ps[:],
                        mybir.ActivationFunctionType.Exp,
                        bias=c_sb[:, mc:mc + 1],
                    )

            def mm2_batch(sg):
                """Z for supergroup sg (blocks 4sg..4sg+3) into one
                [128, 1024] PSUM tile; 4 blocks via col-tile_position run
                concurrently in the PE array."""
                ps_z = pszp.tile([4 * 32, BLK], f32, tag="ps_z", name="ps_z")
                psz_of[sg] = ps_z
                for half in range(2):
                    for mc in range(NCHUNK):
                        for bgl in range(4):
                            e_t = e_of[sg, mc]
                            nc.tensor.matmul(
                                ps_z[32 * bgl:32 * (bgl + 1),
                                     half * FMAX:(half + 1) * FMAX],
                                g_sb[:, mc * 32:(mc + 1) * 32],
                                e_t[:, bgl * BLK + half * FMAX:
                                    bgl * BLK + (half + 1) * FMAX],
                                start=(mc == 0), stop=(mc == NCHUNK - 1),
                                tile_position=(0, 32 * bgl),
                                skip_group_check=True,
                            )

            def emit_recip(sg, q):
                # 1/Z = exp(-ln Z) on ACT (Ln and Exp share one table set);
                # the DVE's iterative-divide reciprocal costs ~6.3 cyc/elem
                # and the DVE queue is the critical path.
                QW = BLK // 2
                if q == 0:
                    rec_of[sg] = rpool.tile([4 * 32, BLK], f16, tag="rec",
                                            name="rec")
                rec = rec_of[sg]
                ps_z = psz_of[sg]
                lnz = lpool.tile([4 * 32, QW], f32, tag="lnz", name="lnz")
                nc.scalar.activation(
                    lnz[:], ps_z[:, q * QW:(q + 1) * QW],
                    mybir.ActivationFunctionType.Ln,
                )
                with nc.allow_low_precision(
                    reason="fp16 rounding of 1/Z feeding the broadcast "
                    "matmul; well within output tolerance"
                ):
                    nc.scalar.activation(
                        rec[:, q * QW:(q + 1) * QW], lnz[:],
                        mybir.ActivationFunctionType.Exp,
                        scale=-1.0,
                    )
                if q == 1:
                    del psz_of[sg]

            outq = []

            def mul_unit(sg, mc):
                """PE broadcasts 1/Z rows onto the chunk's 128 partitions
                (PSUM, 4-way row-packed across the 4 blocks), then DVE
                multiplies e * (1/Z) at 1x from PSUM.  The output DMA is
                emitted two units later so its wait-on-mul is pre-satisfied
                and never head-blocks the Sync queue."""
                rec = rec_of[sg]
                e_t = e_of.pop((sg, mc))
                o_t = opool.tile([MC, 2 * SBW], f16, tag="o_t", name="o_t")
                for h in range(2):
                    for bgl in range(4):
                        ps_b = psbp.tile([MC, FMAX], f32, tag="ps_b",
                                         name="ps_b")
                        nc.tensor.matmul(
                            ps_b[:],
                            b_sb[32 * bgl:32 * (bgl + 1),
                                 mc * MC:(mc + 1) * MC],
                            rec[32 * bgl:32 * (bgl + 1),
                                h * FMAX:(h + 1) * FMAX],
                            start=True, stop=True,
                            tile_position=(32 * bgl, 0),
                        )
                        seg = bgl * BLK + h * FMAX
                        nc.vector.tensor_mul(
                            o_t[:, seg:seg + FMAX],
                            e_t[:, seg:seg + FMAX],
                            ps_b[:],
                        )
                outq.append((sg, mc, o_t))
                while len(outq) > 2:
                    emit_out()

            def emit_out():
                sg, mc, o_t = outq.pop(0)
                nc.sync.dma_start(
                    y_out[mc * MC:(mc + 1) * MC,
                          sg * 2 * SBW:(sg + 1) * 2 * SBW],
                    o_t[:],
                )

            pending = []
            fetch_x(0)
            fetch_x(1)
            def drain(n):
                for _ in range(min(n, len(pending))):
                    mul_unit(*pending.pop(0))

            for j in range(NSB + 5):
                if j < NSB:
                    fetch_x(j + 2)
                    for mc in range(NCHUNK):
                        mm1_exp(j, mc)
                        # fine-grained tail drain: 2 units after every 4
                        # chunks keeps the PE's psb ping-pong stalls short
                        # so the ACT exp stream never starves behind them
                        if mc % 4 == 3:
                            drain(2)
                else:
                    drain(4)
                if j <= NSB and j % 2 == 1:
                    sg = (j - 1) // 2
                    mm2_batch(sg)
                    for q in range(2):
                        emit_recip(sg, q)
                    for mc in range(NCHUNK):
                        pending.append((sg, mc))
            drain(len(pending))
            assert not pending, len(pending)
            while outq:
                emit_out()

    _split_excess_waits(nc)
    return nc


def _prep_core_inputs(x, normals, offsets):
    f16 = np.float16
    bias = np.einsum("md,md->m", normals, offsets).astype(np.float32)
    w_bf = np.ascontiguousarray(normals.T).astype(f16)

    gid = np.arange(M) // GROUP                     # global group of each m
    gsum = np.zeros((MC, NCHUNK * 32), np.float32)
    for mc in range(NCHUNK):
        for p in range(MC):
            r = gid[mc * MC + p] % 32               # group-within-block row
            gsum[p, mc * 32 + r] = 1.0
    gsum = gsum.astype(f16)
    gbc = np.zeros((32, NCHUNK * MC), np.float32)
    for mc in range(NCHUNK):
        for p in range(MC):
            r = gid[mc * MC + p] % 32
            gbc[r, mc * MC + p] = 1.0
    gbc = np.tile(gbc, (4, 1)).astype(f16)
    bias_neg = np.ascontiguousarray(
        -bias.reshape(NCHUNK, MC).T
    ).astype(np.float32)

    in_maps = []
    for b in range(NCORES):
        x_bf = np.ascontiguousarray(x[b].reshape(D, PIX)).astype(f16)
        in_maps.append(
            {"x_bf": x_bf, "normals_bf": w_bf, "gsum_w": gsum,
             "gbc_w": gbc, "bias_neg": bias_neg}
        )
    return in_maps


def _sibmat_is_expected(sibmat):
    gid = np.arange(M) // GROUP
    expected = (gid[:, None] == gid[None, :]).astype(np.float32)
    return sibmat.shape == (M, M) and np.array_equal(sibmat, expected)


def _numpy_fallback(x, normals, offsets, sibmat):
    # Straight fp32 transcription of the reference; only used if sibmat is
    # not the expected 32-wide block-diagonal matrix.
    bias = np.einsum("md,md->m", normals, offsets)
    out = np.empty((B, M, H, W), np.float32)
    for b in range(B):
        logits = np.einsum("dhw,md->mhw", x[b], normals) - bias[:, None, None]
        logits -= np.max(logits, axis=0, keepdims=True)
        e = np.exp(logits)
        z = np.einsum("mhw,nm->nhw", e, sibmat)
        out[b] = e / (z + 1e-15)
    return out


def kernel(x, normals, offsets, sibmat, steps=None, trace=False, **_ignored):
    from concourse.bass_utils import run_bass_kernel_spmd

    x = np.ascontiguousarray(np.asarray(x, dtype=np.float32))
    normals = np.ascontiguousarray(np.asarray(normals, dtype=np.float32))
    offsets = np.ascontiguousarray(np.asarray(offsets, dtype=np.float32))
    sibmat = np.ascontiguousarray(np.asarray(sibmat, dtype=np.float32))

    if (
        not _sibmat_is_expected(sibmat)
        or np.abs(normals).max() > 0.5
        or np.abs(x).max() > 12.0
    ):
        # unexpected structure or value ranges outside the fp16-safe
        # envelope of the device kernel: compute exactly on host
        return _numpy_fallback(x, normals, offsets, sibmat)

    if "nc" not in _cache:
        _cache["nc"] = _build_nc()
    nc = _cache["nc"]

    in_maps = _prep_core_inputs(x, normals, offsets)
    res = run_bass_kernel_spmd(nc, in_maps, list(range(NCORES)), trace=trace)
    out = np.stack(
        [np.asarray(r["y"]).astype(np.float32).reshape(M, H, W)
         for r in res.results]
    )
    kernel.last_result = res
    return out


# revision 11
# speedup vs baseline: 1.9650x; 1.0013x over previous
"""Trainium2 Bass kernel for hierarchical (sibling-group) softmax over
hyperplane margins.

For x:(8,64,128,128), normals/offsets:(1024,64), sibmat block-diagonal with
32-wide sibling groups:

    logits[b,m,h,w] = <x[b,:,h,w], normals[m]> - <normals[m], offsets[m]>
    out = exp(logits) / (group_sum_32(exp(logits)) + 1e-15)

Sharding: data-parallel over batch, one batch element per NeuronCore (8
cores), no collectives.  Per core, m-chunks of 128 rows live on partitions
and pixels on the free axis.

Design (v3, measured on HW; 361 us vs 446 us for the previous version):
  1. PE matmul fp16: logits = normals.T @ x into fp32 PSUM, 4 x N=512 MMs
     per (superblock, chunk) sharing one LDWEIGHTS; warm PE streams these
     at ~216-330 ns/MM (2.4 GHz) because the chains carry few semaphore
     waits.
  2. ACT exp PSUM->SBUF fp16 (N=1024 per instruction) into [128, 4096]
     supergroup-wide e tiles; the per-partition bias argument supplies
     -<normals_m, offsets_m> exactly in fp32.
  3. PE matmul with per-chunk [128, 32] group indicators accumulates Z for
     4 blocks into one [128, 1024] PSUM tile; the 4 blocks' matmuls use
     col-tile_position 32*bg, so quads execute concurrently in the PE's
     32x32 sub-arrays (~3.7x measured).
  4. 1/Z = exp(-ln Z) on ACT (Ln+Exp share one table set; the DVE's
     iterative-divide reciprocal is ~6.3 cyc/elem and DVE is the critical
     engine), written as an fp16 [128, 1024] tile per supergroup.
  5. PE matmul with [32, 128] indicators broadcasts the 1/Z rows onto each
     chunk's 128 partitions (fp32 PSUM, 4-way row-tile_position packed
     across bands), and DVE tensor_tensor multiplies e * (1/Z) at 1x from
     PSUM into fp16 [128, 4096] output tiles.
  6. 512 KB output DMAs land directly in the reference (M, H, W) layout;
     the host upcasts fp16 -> fp32.

Rejected alternative (measured): replicating 1/Z via SBUF->SBUF DMAs with
a zero-step source AP (to run the multiply at DVE 2x from SBUF) works but
is unshippable — the broadcast AP costs ~6.2 us per issue on the HWDGE
queue and ~10 us completion latency via SWDGE, serializing the tail.

A post-pass splits multi-wait instructions (walrus's TRN2 codegen encodes
at most one semaphore wait per compute instruction).  fp16 keeps all
value ranges exact to ~1.8e-3 of the fp32 reference (guarded by
input-range checks that fall back to an exact host implementation).
"""

import numpy as np

B, D, H, W = 8, 64, 128, 128
M = 1024
GROUP = 32
PIX = H * W          # 16384 pixels per batch element
BLK = 1024           # pixels per block
SBW = 2048           # pixels per superblock (2 blocks)
NSB = PIX // SBW     # 8 superblocks
MC = 128             # m-chunk width (partition dim)
NCHUNK = M // MC     # 8
NCORES = 8
FMAX = 512           # max moving free dim into one PSUM bank (fp32 out)
NSG = 4              # supergroups (4 blocks each) per core

_cache = {}


_WAIT_OK_OPCODES = {"Call"}


def _split_excess_waits(nc):
    """Walrus's TRN2 codegen (CoreV3GenImpl setupSyncWait) encodes at most
    one semaphore wait per compute instruction (Matmult, TensorTensor, ...);
    Tile can legitimately attach several (e.g. waits on two input DMAs).
    Move all but one wait onto EventSemaphore instructions inserted just
    before the instruction on the same engine — ordering is identical."""
    import concourse.mybir as mybir

    n_fixed = 0
    for f in nc.m.functions:
        for blk in f.blocks:
            out = []
            changed = False
            for inst in blk.instructions:
                si = inst.sync_info
                if (
                    si is not None
                    and len(si.on_wait) > 1
                    and inst.opcode not in _WAIT_OK_OPCODES
                ):
                    waits = list(si.on_wait)
                    for j, w in enumerate(waits[:-1]):
                        out.append(
                            mybir.InstEventSemaphore(
                                name=f"{inst.name}-wsplit{j}",
                                opcode="EventSemaphore",
                                engine=inst.engine,
                                sync_info=mybir.SyncInfo(
                                    on_wait=[w], on_update=[]
                                ),
                            )
                        )
                    inst.sync_info = mybir.SyncInfo(
                        on_wait=[waits[-1]], on_update=list(si.on_update)
                    )
                    changed = True
                    n_fixed += 1
                out.append(inst)
            if changed:
                blk.instructions = out
    return n_fixed


def _build_nc():
    import concourse.bass as bass
    import concourse.mybir as mybir
    import concourse.tile as tile

    f32 = mybir.dt.float32
    f16 = mybir.dt.float16
    nc = bass.Bass()

    x_in = nc.declare_dram_parameter("x_bf", [D, PIX], f16, isOutput=False)
    w_in = nc.declare_dram_parameter("normals_bf", [D, M], f16, isOutput=False)
    # gsum_w[:, mc*32:(mc+1)*32]: [128, 32] indicator, [p, r] = 1 iff
    # r == 4*mc + p//32 — maps chunk mc's partitions to its 32 global
    # groups-within-block rows of the Z tile.
    g_in = nc.declare_dram_parameter("gsum_w", [MC, NCHUNK * 32], f16,
                                     isOutput=False)
    # bias_neg[p, mc] = -<normals, offsets> for m = mc*128+p; applied as the
    # ACT exp per-partition bias (exact fp32).
    # gbc_w[32*q + r, mc*128 + p]: replica q of the [32, 128] indicator that
    # broadcasts group row 4*mc + p//32 onto chunk partitions; replicas let
    # mm3's stationary start at the same partition base as its moving
    # operand (rows 32*bgl of the rec tile).
    b_in = nc.declare_dram_parameter("gbc_w", [4 * 32, NCHUNK * MC], f16,
                                     isOutput=False)
    c_in = nc.declare_dram_parameter("bias_neg", [MC, NCHUNK], f32,
                                     isOutput=False)
    y_out = nc.declare_dram_parameter("y", [M, PIX], f16, isOutput=True)

    with tile.TileContext(nc) as tc:
        with (
            tc.tile_pool(name="const", bufs=1) as cpool,
            tc.tile_pool(name="xin", bufs=3) as xpool,
            tc.tile_pool(name="expv", bufs=16) as epool,
            tc.tile_pool(name="recv", bufs=2) as rpool,
            tc.tile_pool(name="lnzv", bufs=2) as lpool,
            tc.tile_pool(name="outv", bufs=4) as opool,
            tc.tile_pool(name="psl", bufs=2, space="PSUM") as pslp,
            tc.tile_pool(name="psz", bufs=1, space="PSUM") as pszp,
            tc.tile_pool(name="psb", bufs=2, space="PSUM") as psbp,
        ):
            w_sb = cpool.tile([D, M], f16)
            nc.sync.dma_start(w_sb[:], w_in[:])
            g_sb = cpool.tile([MC, NCHUNK * 32], f16)
            nc.sync.dma_start(g_sb[:], g_in[:])
            b_sb = cpool.tile([4 * 32, NCHUNK * MC], f16)
            nc.sync.dma_start(b_sb[:], b_in[:])
            c_sb = cpool.tile([MC, NCHUNK], f32)
            nc.sync.dma_start(c_sb[:], c_in[:])

            x_of = {}
            e_of = {}
            rec_of = {}
            psz_of = {}

            def fetch_x(sb):
                if sb in x_of or sb >= NSB:
                    return
                x_t = xpool.tile([D, SBW], f16, tag="x_t", name="x_t")
                nc.sync.dma_start(x_t[:], x_in[:, sb * SBW:(sb + 1) * SBW])
                x_of[sb] = x_t

            def mm1_exp(sb, mc):
                """logits + exp for chunk mc of superblock sb (2 blocks).
                e tiles span a whole supergroup [128, 4096] so the tail
                (broadcast/multiply/out-DMA) works in coarse units."""
                x_t = x_of[sb]
                sg = sb // 2
                if sb % 2 == 0:
                    e_of[sg, mc] = epool.tile([MC, 2 * SBW], f16,
                                              tag="e_t", name="e_t")
                e_t = e_of[sg, mc]
                for b in range(2):          # block within superblock
                    cb = (sb % 2) * SBW + b * BLK
                    ps = pslp.tile([MC, BLK], f32, tag="ps_l", name="ps_l")
                    for h in range(2):
                        nc.tensor.matmul(
                            ps[:, h * FMAX:(h + 1) * FMAX],
                            w_sb[:, mc * MC:(mc + 1) * MC],
                            x_t[:, b * BLK + h * FMAX:
                                b * BLK + (h + 1) * FMAX],
                            start=True, stop=True,
                        )
                    nc.scalar.activation(
                        e_t[:, cb:cb + BLK], ps[:],
                        mybir.ActivationFunctionType.Exp,
                        bias=c_sb[:, mc:mc + 1],
                    )

            def mm2_batch(sg):
                """Z for supergroup sg (blocks 4sg..4sg+3) into one
                [128, 1024] PSUM tile; 4 blocks via col-tile_position run
                concurrently in the PE array."""
                ps_z = pszp.tile([4 * 32, BLK], f32, tag="ps_z", name="ps_z")
                psz_of[sg] = ps_z
                for half in range(2):
                    for mc in range(NCHUNK):
                        for bgl in range(4):
                            e_t = e_of[sg, mc]
                            nc.tensor.matmul(
                                ps_z[32 * bgl:32 * (bgl + 1),
                                     half * FMAX:(half + 1) * FMAX],
                                g_sb[:, mc * 32:(mc + 1) * 32],
                                e_t[:, bgl * BLK + half * FMAX:
                                    bgl * BLK + (half + 1) * FMAX],
                                start=(mc == 0), stop=(mc == NCHUNK - 1),
                                tile_position=(0, 32 * bgl),
                                skip_group_check=True,
                            )

            def emit_recip(sg, q):
                # 1/Z = exp(-ln Z) on ACT (Ln and Exp share one table set);
                # the DVE's iterative-divide reciprocal costs ~6.3 cyc/elem
                # and the DVE queue is the critical path.
                QW = BLK // 2
                if q == 0:
                    rec_of[sg] = rpool.tile([4 * 32, BLK], f16, tag="rec",
                                            name="rec")
                rec = rec_of[sg]
                ps_z = psz_of[sg]
                lnz = lpool.tile([4 * 32, QW], f32, tag="lnz", name="lnz")
                nc.scalar.activation(
                    lnz[:], ps_z[:, q * QW:(q + 1) * QW],
                    mybir.ActivationFunctionType.Ln,
                )
                with nc.allow_low_precision(
                    reason="fp16 rounding of 1/Z feeding the broadcast "
                    "matmul; well within output tolerance"
                ):
                    nc.scalar.activation(
                        rec[:, q * QW:(q + 1) * QW], lnz[:],
                        mybir.ActivationFunctionType.Exp,
                        scale=-1.0,
                    )
                if q == 1:
                    del psz_of[sg]

            outq = []

            def mul_unit(sg, mc):
                """PE broadcasts 1/Z rows onto the chunk's 128 partitions
                (PSUM, 4-way row-packed across the 4 blocks), then DVE
                multiplies e * (1/Z) at 1x from PSUM.  The output DMA is
                emitted two units later so its wait-on-mul is pre-satisfied
                and never head-blocks the Sync queue."""
                rec = rec_of[sg]
                e_t = e_of.pop((sg, mc))
                o_t = opool.tile([MC, 2 * SBW], f16, tag="o_t", name="o_t")
                for h in range(2):
                    for bgl in range(4):
                        ps_b = psbp.tile([MC, FMAX], f32, tag="ps_b",
                                         name="ps_b")
                        nc.tensor.matmul(
                            ps_b[:],
                            b_sb[32 * bgl:32 * (bgl + 1),
                                 mc * MC:(mc + 1) * MC],
                            rec[32 * bgl:32 * (bgl + 1),
                                h * FMAX:(h + 1) * FMAX],
                            start=True, stop=True,
                            tile_position=(32 * bgl, 0),
                        )
                        seg = bgl * BLK + h * FMAX
                        nc.vector.tensor_mul(
                            o_t[:, seg:seg + FMAX],
                            e_t[:, seg:seg + FMAX],
                            ps_b[:],
                        )
                outq.append((sg, mc, o_t))
                while len(outq) > 2:
                    emit_out()

            def emit_out():
                sg, mc, o_t = outq.pop(0)
                nc.sync.dma_start(
                    y_out[mc * MC:(mc + 1) * MC,
                          sg * 2 * SBW:(sg + 1) * 2 * SBW],
                    o_t[:],
                )

            pending = []
            fetch_x(0)
            fetch_x(1)
            def drain(n):
                for _ in range(min(n, len(pending))):
                    mul_unit(*pending.pop(0))

            for j in range(NSB + 5):
                if j < NSB:
                    fetch_x(j + 2)
                    for mc in range(NCHUNK):
                        mm1_exp(j, mc)
                        # fine-grained tail drain: 2 units after every 4
                        # chunks keeps the PE's psb ping-pong stalls short
                        # so the ACT exp stream never starves behind them
                        if mc % 4 == 3:
                            drain(2)
                else:
                    drain(4)
                if j <= NSB and j % 2 == 1:
                    sg = (j - 1) // 2
                    mm2_batch(sg)
                    for q in range(2):
                        emit_recip(sg, q)
                    for mc in range(NCHUNK):
                        pending.append((sg, mc))
            drain(len(pending))
            assert not pending, len(pending)
            while outq:
                emit_out()

    _split_excess_waits(nc)
    return nc


def _prep_core_inputs(x, normals, offsets):
    f16 = np.float16
    bias = np.einsum("md,md->m", normals, offsets).astype(np.float32)
    w_bf = np.ascontiguousarray(normals.T).astype(f16)

    gid = np.arange(M) // GROUP                     # global group of each m
    gsum = np.zeros((MC, NCHUNK * 32), np.float32)
    for mc in range(NCHUNK):
        for p in range(MC):
            r = gid[mc * MC + p] % 32               # group-within-block row
            gsum[p, mc * 32 + r] = 1.0
    gsum = gsum.astype(f16)
    gbc = np.zeros((32, NCHUNK * MC), np.float32)
    for mc in range(NCHUNK):
        for p in range(MC):
            r = gid[mc * MC + p] % 32
            gbc[r, mc * MC + p] = 1.0
    gbc = np.tile(gbc, (4, 1)).astype(f16)
    bias_neg = np.ascontiguousarray(
        -bias.reshape(NCHUNK, MC).T
    ).astype(np.float32)

    in_maps = []
    for b in range(NCORES):
        x_bf = np.ascontiguousarray(x[b].reshape(D, PIX)).astype(f16)
        in_maps.append(
            {"x_bf": x_bf, "normals_bf": w_bf, "gsum_w": gsum,
             "gbc_w": gbc, "bias_neg": bias_neg}
        )
    return in_maps


def _sibmat_is_expected(sibmat):
    gid = np.arange(M) // GROUP
    expected = (gid[:, None] == gid[None, :]).astype(np.float32)
    return sibmat.shape == (M, M) and np.array_equal(sibmat, expected)


def _numpy_fallback(x, normals, offsets, sibmat):
    # Straight fp32 transcription of the reference; only used if sibmat is
    # not the expected 32-wide block-diagonal matrix.
    bias = np.einsum("md,md->m", normals, offsets)
    out = np.empty((B, M, H, W), np.float32)
    for b in range(B):
        logits = np.einsum("dhw,md->mhw", x[b], normals) - bias[:, None, None]
        logits -= np.max(logits, axis=0, keepdims=True)
        e = np.exp(logits)
        z = np.einsum("mhw,nm->nhw", e, sibmat)
        out[b] = e / (z + 1e-15)
    return out


def kernel(x, normals, offsets, sibmat, steps=None, trace=False, **_ignored):
    from concourse.bass_utils import run_bass_kernel_spmd

    x = np.ascontiguousarray(np.asarray(x, dtype=np.float32))
    normals = np.ascontiguousarray(np.asarray(normals, dtype=np.float32))
    offsets = np.ascontiguousarray(np.asarray(offsets, dtype=np.float32))
    sibmat = np.ascontiguousarray(np.asarray(sibmat, dtype=np.float32))

    if (
        not _sibmat_is_expected(sibmat)
        or np.abs(normals).max() > 0.5
        or np.abs(x).max() > 12.0
    ):
        # unexpected structure or value ranges outside the fp16-safe
        # envelope of the device kernel: compute exactly on host
        return _numpy_fallback(x, normals, offsets, sibmat)

    if "nc" not in _cache:
        _cache["nc"] = _build_nc()
    nc = _cache["nc"]

    in_maps = _prep_core_inputs(x, normals, offsets)
    res = run_bass_kernel_spmd(nc, in_maps, list(range(NCORES)), trace=trace)
    out = np.stack(
        [np.asarray(r["y"]).astype(np.float32).reshape(M, H, W)
         for r in res.results]
    )
    kernel.last_result = res
    return out
